# revision 1
# baseline (speedup 1.0000x reference)
"""Trainium2 Bass kernel for nn_CosmosPatcher3d.

Computes the Cosmos 3D Haar wavelet patcher: input [1,3,33,704,704] fp32,
temporal causal pad (first frame repeated 4x -> 36 frames), then two full
3D Haar DWT levels. Equivalent to a separable +-1 Hadamard transform over
4x4x4 blocks scaled by 1/64, producing [1,192,9,176,176] fp32 with channel
layout ch = 96*T2 + 48*H2 + 24*W2 + 12*T1 + 6*H1 + 3*W1 + c.

Strategy (8 NeuronCores, shard along H: 704 = 8*88, 4-blocks don't straddle):
- Host splits x into bf16 hi/lo (x ~= hi + lo to ~2^-18 relative) so TensorE
  runs at bf16 rate while accumulating both halves into fp32 PSUM: exact to
  ~1e-5. Same HBM bytes as fp32.
- TensorE: fused T+H transform. lhsT = +-1/64 sign matrix [K=(dt,hh),
  M=(TH2, y', TH1)], rhs = [K, (hi|lo, w)] per channel.
- ScalarE: PSUM -> SBUF copies. VectorE: two-level strided W butterfly,
  output free layout (W2, W1, c, x').
- Stores: per (TH2, W2) -> HBM AP [y', (TH1,W1,c) merged 24 @ ch-stride, x]
  = 3 dims, 8 stores x 135KB per (t, chunk).
"""

import ml_dtypes
import numpy as np

import concourse.bacc as bacc
import concourse.mybir as mybir
import concourse.tile as tile
from concourse.bass_utils import run_bass_kernel_spmd

N_CORES = 8
C = 3            # input channels
T_IN = 33        # input frames
H_IN = 704       # input height (global)
W_IN = 704       # input width
H_SH = H_IN // N_CORES      # 88 input rows per core
T_OUT = 9
Y_SH = H_SH // 4            # 22 output rows per core
X_OUT = W_IN // 4           # 176
FREE = C * W_IN             # 2112
CHUNKS = [(0, 32), (32, 32), (64, 24)]

_F32 = mybir.dt.float32
_BF16 = mybir.dt.bfloat16
_BF16_NP = ml_dtypes.bfloat16


def _sgn1d(pos, b2, b1):
    """Composite 2-level Haar sign for position pos in 0..3 (+-1)."""
    s1 = 1.0 if b1 == 0 else (1.0 - 2.0 * (pos % 2))
    s2 = 1.0 if b2 == 0 else (1.0 - 2.0 * (pos // 2))
    return s1 * s2


def _build_signs():
    """bf16 sign matrices including the global 1/64 scale (exact in bf16).

    M ordering: m = (T2*2+H2)*32 + y'*4 + (T1*2+H1)   (y' = hh//4).
    s32 [128,128]: rows dt*32+hh, t>=1 chunks of 32 rows.
    s24 [96,128]:  rows dt*24+hh, 24-row chunk (y'>5 columns zero).
    t32 [32,128] / t24 [24,128]: t=0 (frame 0 repeated 4x -> only T2=T1=0
    subbands nonzero, weight 4).
    """
    def mk(nh, t0):
        k = nh if t0 else 4 * nh
        s = np.zeros((k, 128), dtype=np.float32)
        for hh in range(nh):
            yp, hp = hh // 4, hh % 4
            for t2 in range(2):
                for h2 in range(2):
                    for t1 in range(2):
                        for h1 in range(2):
                            col = (t2 * 2 + h2) * 32 + yp * 4 + (t1 * 2 + h1)
                            sh = _sgn1d(hp, h2, h1)
                            if t0:
                                if t2 == 0 and t1 == 0:
                                    s[hh, col] = 4.0 * sh / 64.0
                            else:
                                for dt in range(4):
                                    st = _sgn1d(dt, t2, t1)
                                    s[dt * nh + hh, col] = st * sh / 64.0
        return s.astype(_BF16_NP)

    return mk(32, False), mk(24, False), mk(32, True), mk(24, True)


def _build_nc():
    nc = bacc.Bacc(
        "TRN2", target_bir_lowering=False, debug=False, num_devices=N_CORES
    )
    # x split hi/lo: [C, T, H, 2, W] bf16
    x = nc.dram_tensor(
        "x", [C, T_IN, H_SH, 2, W_IN], _BF16, kind="ExternalInput"
    ).ap()
    s32 = nc.dram_tensor("s32", [128, 128], _BF16, kind="ExternalInput").ap()
    s24 = nc.dram_tensor("s24", [96, 128], _BF16, kind="ExternalInput").ap()
    t32 = nc.dram_tensor("t32", [32, 128], _BF16, kind="ExternalInput").ap()
    t24 = nc.dram_tensor("t24", [24, 128], _BF16, kind="ExternalInput").ap()
    out = nc.dram_tensor(
        "out", [192, T_OUT, Y_SH, X_OUT], _F32, kind="ExternalOutput"
    ).ap()

    # Store view: ch = 48*TH2 + 24*W2 + (6*TH1 + 3*W1 + c).
    # Per (TH2, W2) store: HBM dims [y', i(24) @ ch-stride, x] -> 3 dims.
    o_v = out.rearrange("(a b i) t y x -> a b t y i x", a=4, b=2, i=24)

    with tile.TileContext(nc) as tc:
        with (
            tc.tile_pool(name="signs", bufs=1) as sgp,
            tc.tile_pool(name="rhs", bufs=6) as rhp,
            tc.tile_pool(name="sbf", bufs=4) as fbp,
            tc.tile_pool(name="sbsd", bufs=4) as sdp,
            tc.tile_pool(name="outp", bufs=6) as otp,
            tc.tile_pool(name="psum", bufs=4, space="PSUM") as psp,
        ):
            ts32 = sgp.tile([128, 128], _BF16)
            ts24 = sgp.tile([96, 128], _BF16)
            tt32 = sgp.tile([32, 128], _BF16)
            tt24 = sgp.tile([24, 128], _BF16)
            nc.sync.dma_start(out=ts32, in_=s32)
            nc.sync.dma_start(out=ts24, in_=s24)
            nc.sync.dma_start(out=tt32, in_=t32)
            nc.sync.dma_start(out=tt24, in_=t24)

            store_i = 0
            for t in range(T_OUT):
                for ci, (h0, nh) in enumerate(CHUNKS):
                    ny = nh // 4
                    kdim = nh if t == 0 else 4 * nh
                    if t == 0:
                        lhsT = tt32 if nh == 32 else tt24
                    else:
                        lhsT = ts32 if nh == 32 else ts24

                    # rhs free layout per c: (hl(2), w)
                    rhs = rhp.tile([128, C, 2 * W_IN], _BF16, tag="rhs")
                    sbf = fbp.tile([128, FREE], _F32, tag="sbf")
                    for c in range(C):
                        if t == 0:
                            src = x[c, 0, h0 : h0 + nh, :, :]
                        else:
                            src = x[c, 4 * t - 3 : 4 * t + 1, h0 : h0 + nh, :, :]
                        nc.sync.dma_start(out=rhs[:kdim, c, :], in_=src)
                        ps = psp.tile([128, W_IN], _F32, tag="ps")
                        for j in range(0, W_IN, 512):
                            n = min(512, W_IN - j)
                            nc.tensor.matmul(
                                ps[:, j : j + n],
                                lhsT,
                                rhs[:kdim, c, j : j + n],
                                start=True,
                                stop=False,
                            )
                            nc.tensor.matmul(
                                ps[:, j : j + n],
                                lhsT,
                                rhs[:kdim, c, W_IN + j : W_IN + j + n],
                                start=False,
                                stop=True,
                            )
                        nc.scalar.copy(
                            out=sbf[:, c * W_IN : (c + 1) * W_IN],
                            in_=ps,
                        )

                    # W level 1: pairs along w -> sums (W1=0) and diffs (W1=1)
                    sbsd = sdp.tile([128, FREE], _F32, tag="sbsd")
                    v = sbf.rearrange("q (c w par) -> q c w par", c=C, par=2)
                    s_half = sbsd[:, : FREE // 2].rearrange("q (c w) -> q c w", c=C)
                    d_half = sbsd[:, FREE // 2 :].rearrange("q (c w) -> q c w", c=C)
                    nc.vector.tensor_add(
                        out=s_half, in0=v[:, :, :, 0], in1=v[:, :, :, 1]
                    )
                    nc.vector.tensor_sub(
                        out=d_half, in0=v[:, :, :, 0], in1=v[:, :, :, 1]
                    )

                    # W level 2 -> out tile free = (W2, W1, c, x')
                    ot = otp.tile([128, FREE], _F32, tag="ot")
                    ov = ot.rearrange(
                        "q (W2 W1 c xx) -> q W2 W1 c xx", W2=2, W1=2, c=C
                    )
                    vs = sbsd[:, : FREE // 2].rearrange(
                        "q (c xx par) -> q c xx par", c=C, par=2
                    )
                    vd = sbsd[:, FREE // 2 :].rearrange(
                        "q (c xx par) -> q c xx par", c=C, par=2
                    )
                    nc.vector.tensor_add(
                        out=ov[:, 0, 0], in0=vs[:, :, :, 0], in1=vs[:, :, :, 1]
                    )
                    nc.vector.tensor_sub(
                        out=ov[:, 1, 0], in0=vs[:, :, :, 0], in1=vs[:, :, :, 1]
                    )
                    nc.vector.tensor_add(
                        out=ov[:, 0, 1], in0=vd[:, :, :, 0], in1=vd[:, :, :, 1]
                    )
                    nc.vector.tensor_sub(
                        out=ov[:, 1, 1], in0=vd[:, :, :, 0], in1=vd[:, :, :, 1]
                    )

                    # stores: one per (TH2, W2); partitions th2*32 + y'*4 + th1
                    y0 = h0 // 4
                    for th2 in range(4):
                        for w2 in range(2):
                            dst = o_v[th2, w2, t, y0 : y0 + ny]
                            # loads own Sync; stores rotate all three rings
                            eng = [nc.sync, nc.scalar, nc.gpsimd][store_i % 3]
                            store_i += 1
                            eng.dma_start(
                                out=dst,
                                in_=ot[
                                    th2 * 32 : th2 * 32 + 4 * ny,
                                    w2 * 1056 : (w2 + 1) * 1056,
                                ],
                            )

    nc.compile()
    return nc


_NC_CACHE = None


def _prep_inputs(hs):
    """Shard along H and split into bf16 hi/lo, interleaved as [..., 2, W]."""
    s32, s24, t32, t24 = _build_signs()
    in_maps = []
    for k in range(N_CORES):
        xk = np.ascontiguousarray(hs[0, :, :, k * H_SH : (k + 1) * H_SH, :])
        hi = xk.astype(_BF16_NP)
        lo = (xk - hi.astype(np.float32)).astype(_BF16_NP)
        xhl = np.stack([hi, lo], axis=3)  # [C, T, H, 2, W]
        in_maps.append(
            {"x": xhl, "s32": s32, "s24": s24, "t32": t32, "t24": t24}
        )
    return in_maps


def kernel(hidden_states: np.ndarray) -> np.ndarray:
    global _NC_CACHE
    if _NC_CACHE is None:
        _NC_CACHE = _build_nc()
    nc = _NC_CACHE

    hs = np.asarray(hidden_states, dtype=np.float32)
    assert hs.shape == (1, C, T_IN, H_IN, W_IN), hs.shape
    in_maps = _prep_inputs(hs)

    res = run_bass_kernel_spmd(nc, in_maps, core_ids=list(range(N_CORES)))

    out = np.empty((1, 192, T_OUT, H_IN // 4, X_OUT), dtype=np.float32)
    for k in range(N_CORES):
        out[0, :, :, k * Y_SH : (k + 1) * Y_SH, :] = res.results[k]["out"]
    return out



# revision 10
# speedup vs baseline: 1.9444x; 1.9444x over previous
"""Trainium2 Bass kernel for nn_CosmosPatcher3d.

Computes the Cosmos 3D Haar wavelet patcher: input [1,3,33,704,704] fp32,
temporal causal pad (first frame repeated 4x -> 36 frames), then two full
3D Haar DWT levels. Equivalent to a separable +-1 Hadamard transform over
4x4x4 blocks scaled by 1/64, producing [1,192,9,176,176] fp32 with channel
layout ch = 96*T2 + 48*H2 + 24*W2 + 12*T1 + 6*H1 + 3*W1 + c.

v2 strategy (8 NeuronCores, shard along H: 704 = 8*88):
- Host converts x to single bf16 (err ~2e-3 << 2e-2 budget): halves load
  bytes vs fp32/hi+lo.
- Loads: partition k = hh*4+dt so the HBM-side AP outer dim is h (24..32
  blocks) -> HWDGE spreads descriptors over all 16 SDMA engines.
- TensorE: fused T+H transform AND W-level-1 butterfly: two PSUM
  accumulation groups per channel; sums use lhsT=S on even/odd stride-2
  rhs slices, diffs use S then -S. M = th2*32 + y'*4 + th1.
- VectorE: only W-level-2 butterfly, reading PSUM directly (no scalar
  PSUM->SBUF copy), writing out tile free layout (W2, W1, c, x').
- Stores: per (th2, w2) -> HBM AP [y', i(24) @ ch-stride, x]; split
  across gpsimd (SWDGE: even 16-engine spread) and sync/scalar (HWDGE:
  engines 64+block).
"""

import ml_dtypes
import numpy as np

import concourse.bacc as bacc
import concourse.mybir as mybir
import concourse.tile as tile
from concourse.bass_utils import run_bass_kernel_spmd

N_CORES = 8
C = 3            # input channels
T_IN = 33        # input frames
H_IN = 704       # input height (global)
W_IN = 704       # input width
H_SH = H_IN // N_CORES      # 88 input rows per core
T_OUT = 9
Y_SH = H_SH // 4            # 22 output rows per core
X_OUT = W_IN // 4           # 176
XH = W_IN // 2              # 352 = level-1 output width
CHUNKS = [(0, 32), (32, 32), (64, 24)]

_F32 = mybir.dt.float32
_BF16 = mybir.dt.bfloat16
_BF16_NP = ml_dtypes.bfloat16


def _sgn1d(pos, b2, b1):
    """Composite 2-level Haar sign for position pos in 0..3 (+-1)."""
    s1 = 1.0 if b1 == 0 else (1.0 - 2.0 * (pos % 2))
    s2 = 1.0 if b2 == 0 else (1.0 - 2.0 * (pos // 2))
    return s1 * s2


def _build_signs():
    """bf16 sign matrices including the global 1/64 scale (exact in bf16).

    Rows k = hh*4 + dt (h-major so the load's HBM AP outer dim is h).
    Cols m = (T2*2+H2)*32 + y'*4 + (T1*2+H1)   (y' = hh//4).
    s32 [128,128] / s24 [96,128]: t>=1.  t32 [32,128] / t24 [24,128]:
    t=0 (frame 0 repeated 4x -> only T2=T1=0 subbands, weight 4).
    """
    def mk(nh, t0):
        k = nh if t0 else 4 * nh
        s = np.zeros((k, 128), dtype=np.float32)
        for hh in range(nh):
            yp, hp = hh // 4, hh % 4
            for t2 in range(2):
                for h2 in range(2):
                    for t1 in range(2):
                        for h1 in range(2):
                            col = (t2 * 2 + h2) * 32 + yp * 4 + (t1 * 2 + h1)
                            sh = _sgn1d(hp, h2, h1)
                            if t0:
                                if t2 == 0 and t1 == 0:
                                    s[hh, col] = 4.0 * sh / 64.0
                            else:
                                for dt in range(4):
                                    st = _sgn1d(dt, t2, t1)
                                    s[hh * 4 + dt, col] = st * sh / 64.0
        return s.astype(_BF16_NP)

    return mk(32, False), mk(24, False), mk(32, True), mk(24, True)


def _build_nc():
    nc = bacc.Bacc(
        "TRN2", target_bir_lowering=False, debug=False, num_devices=N_CORES
    )
    x = nc.dram_tensor(
        "x", [C, T_IN, H_SH, W_IN], _BF16, kind="ExternalInput"
    ).ap()
    sg = {}
    for nm, shp in [
        ("s32", [128, 128]), ("s24", [96, 128]),
        ("t32", [32, 128]), ("t24", [24, 128]),
        ("s32n", [128, 128]), ("s24n", [96, 128]),
        ("t32n", [32, 128]), ("t24n", [24, 128]),
    ]:
        sg[nm] = nc.dram_tensor(nm, shp, _BF16, kind="ExternalInput").ap()
    out = nc.dram_tensor(
        "out", [192, T_OUT, Y_SH, X_OUT], _F32, kind="ExternalOutput"
    ).ap()

    # Store view: ch = 48*TH2 + 24*W2 + (6*TH1 + 3*W1 + c).
    # Per (TH2, W2) store: HBM dims [y', i(24) @ ch-stride, x] -> 3 dims.
    o_v = out.rearrange("(a b i) t y x -> a b t y i x", a=4, b=2, i=24)

    with tile.TileContext(nc) as tc:
        with (
            tc.tile_pool(name="signs", bufs=1) as sgp,
            tc.tile_pool(name="rhs", bufs=4) as rhp,
            tc.tile_pool(name="even", bufs=4) as evp,
            tc.tile_pool(name="outp", bufs=4) as otp,
            tc.tile_pool(name="psum", bufs=2, space="PSUM") as psp,
        ):
            st = {}
            for nm in ("s32", "s24", "t32", "t24", "s32n", "s24n", "t32n", "t24n"):
                t_ = sgp.tile(list(sg[nm].shape), _BF16, tag=nm)
                nc.sync.dma_start(out=t_, in_=sg[nm])
                st[nm] = t_

            store_i = 0
            for t in range(T_OUT):
                for ci, (h0, nh) in enumerate(CHUNKS):
                    ny = nh // 4
                    kdim = nh if t == 0 else 4 * nh
                    if t == 0:
                        lp = st["t32"] if nh == 32 else st["t24"]
                        ln = st["t32n"] if nh == 32 else st["t24n"]
                    else:
                        lp = st["s32"] if nh == 32 else st["s24"]
                        ln = st["s32n"] if nh == 32 else st["s24n"]

                    # loads: partition k = hh*4+dt; HBM AP outer dim = h
                    rhs = rhp.tile([128, C, W_IN], _BF16, tag="rhs")
                    for c in range(C):
                        if t == 0:
                            src = x[c, 0, h0 : h0 + nh, :]
                        else:
                            src = x[c, 4 * t - 3 : 4 * t + 1, h0 : h0 + nh, :]
                            src = src.rearrange("t h w -> h t w")
                        eng = [nc.sync, nc.sync, nc.scalar][c]
                        eng.dma_start(out=rhs[:kdim, c, :], in_=src)

                    rv = rhs.rearrange("k c (xp par) -> k c xp par", par=2)

                    # W-level-1 sums in PE: ps_s[m, c, x'] = S @ (even + odd).
                    # Each c gets a full 512-f32 PSUM bank so accumulation
                    # groups never straddle bank boundaries.
                    ps_s = psp.tile([128, C, 512], _F32, tag="ps")
                    for c in range(C):
                        nc.tensor.matmul(
                            ps_s[:, c, :XH], lp, rv[:kdim, c, :, 0],
                            start=True, stop=False,
                        )
                        nc.tensor.matmul(
                            ps_s[:, c, :XH], lp, rv[:kdim, c, :, 1],
                            start=False, stop=True,
                        )

                    # W-level-2 from sums -> out tile (W2, W1=0, c, x'').
                    # TensorTensor may read only ONE operand from PSUM, so
                    # ScalarE first copies the even-parity slice to SBUF.
                    ot = otp.tile([128, 2, 2, C, X_OUT], _F32, tag="ot")
                    sv = ps_s.rearrange("m c (xx par) -> m c xx par", par=2)
                    se = evp.tile([128, C, X_OUT], _F32, tag="ev")
                    nc.scalar.copy(out=se, in_=sv[:, :, :X_OUT, 0])
                    nc.vector.tensor_add(
                        out=ot[:, 0, 0], in0=se, in1=sv[:, :, :X_OUT, 1]
                    )
                    nc.vector.tensor_sub(
                        out=ot[:, 1, 0], in0=se, in1=sv[:, :, :X_OUT, 1]
                    )

                    # W-level-1 diffs in PE: ps_d = S @ even + (-S) @ odd
                    ps_d = psp.tile([128, C, 512], _F32, tag="ps")
                    for c in range(C):
                        nc.tensor.matmul(
                            ps_d[:, c, :XH], lp, rv[:kdim, c, :, 0],
                            start=True, stop=False,
                        )
                    for c in range(C):
                        nc.tensor.matmul(
                            ps_d[:, c, :XH], ln, rv[:kdim, c, :, 1],
                            start=False, stop=True,
                        )

                    dv = ps_d.rearrange("m c (xx par) -> m c xx par", par=2)
                    de = evp.tile([128, C, X_OUT], _F32, tag="ev")
                    nc.scalar.copy(out=de, in_=dv[:, :, :X_OUT, 0])
                    nc.vector.tensor_add(
                        out=ot[:, 0, 1], in0=de, in1=dv[:, :, :X_OUT, 1]
                    )
                    nc.vector.tensor_sub(
                        out=ot[:, 1, 1], in0=de, in1=dv[:, :, :X_OUT, 1]
                    )

                    # stores: one per (TH2, W2); partitions th2*32 + y'*4 + th1
                    y0 = h0 // 4
                    for th2 in range(4):
                        for w2 in range(2):
                            dst = o_v[th2, w2, t, y0 : y0 + ny]
                            if ny == 6:
                                eng = nc.gpsimd
                            else:
                                eng = [nc.gpsimd, nc.sync, nc.scalar,
                                       nc.scalar][store_i % 4]
                                store_i += 1
                            eng.dma_start(
                                out=dst,
                                in_=ot[
                                    th2 * 32 : th2 * 32 + 4 * ny,
                                    w2, :, :, :,
                                ],
                            )

    nc.compile()
    return nc


_NC_CACHE = None


def _prep_inputs(hs):
    """Shard along H, convert to single bf16."""
    s32, s24, t32, t24 = _build_signs()
    base = {
        "s32": s32, "s24": s24, "t32": t32, "t24": t24,
        "s32n": -s32, "s24n": -s24, "t32n": -t32, "t24n": -t24,
    }
    in_maps = []
    for k in range(N_CORES):
        xk = np.ascontiguousarray(
            hs[0, :, :, k * H_SH : (k + 1) * H_SH, :]
        ).astype(_BF16_NP)
        m = dict(base)
        m["x"] = xk
        in_maps.append(m)
    return in_maps


def kernel(hidden_states: np.ndarray) -> np.ndarray:
    global _NC_CACHE
    if _NC_CACHE is None:
        _NC_CACHE = _build_nc()
    nc = _NC_CACHE

    hs = np.asarray(hidden_states, dtype=np.float32)
    assert hs.shape == (1, C, T_IN, H_IN, W_IN), hs.shape
    in_maps = _prep_inputs(hs)

    res = run_bass_kernel_spmd(nc, in_maps, core_ids=list(range(N_CORES)))

    out = np.empty((1, 192, T_OUT, H_IN // 4, X_OUT), dtype=np.float32)
    for k in range(N_CORES):
        out[0, :, :, k * Y_SH : (k + 1) * Y_SH, :] = res.results[k]["out"]
    return out


# revision 14
# speedup vs baseline: 2.0818x; 1.0707x over previous
"""Trainium2 Bass kernel for nn_CosmosPatcher3d.

Computes the Cosmos 3D Haar wavelet patcher: input [1,3,33,704,704] fp32,
temporal causal pad (first frame repeated 4x -> 36 frames), then two full
3D Haar DWT levels. Equivalent to a separable +-1 Hadamard transform over
4x4x4 blocks scaled by 1/64, producing [1,192,9,176,176] fp32 with channel
layout ch = 96*T2 + 48*H2 + 24*W2 + 12*T1 + 6*H1 + 3*W1 + c.

v4 strategy (8 NeuronCores, shard along H: 704 = 8*88):
- Host converts x to single bf16 (err ~2e-3 << 2e-2 budget) packed as
  [T, H, C, W] so one 3-dim DMA per (t, chunk) loads all channels with
  4224B descriptors, HBM outer dim = h (24..32 blocks -> 16 engines).
- TensorE: fused T+H transform AND W-level-1 butterfly: sums via lhsT=S
  on even/odd stride-2 rhs slices accumulated in PSUM; diffs via S then
  -S. M = th2*32 + y'*4 + th1. Each c gets a full PSUM bank.
- W-level-2: ScalarE/VectorE copy the even-parity PSUM slice to SBUF
  (TensorTensor allows only one PSUM operand), VectorE does add/sub.
- Out tile free layout (w1, c, w2, x) so the store's SBUF-side strides
  (th1@2112, w1@1056, c@352) nest uniformly -> 3-dim AP with HBM outer
  dim i(24): HWDGE spreads stores over all 16 engines. SWDGE (gpsimd)
  takes a share for even balance.
"""

import ml_dtypes
import numpy as np

import concourse.bacc as bacc
import concourse.mybir as mybir
import concourse.tile as tile
from concourse.bass_utils import run_bass_kernel_spmd

N_CORES = 8
C = 3            # input channels
T_IN = 33        # input frames
H_IN = 704       # input height (global)
W_IN = 704       # input width
H_SH = H_IN // N_CORES      # 88 input rows per core
T_OUT = 9
Y_SH = H_SH // 4            # 22 output rows per core
X_OUT = W_IN // 4           # 176
XH = W_IN // 2              # 352 = level-1 output width
CHUNKS = [(0, 32), (32, 32), (64, 24)]

_F32 = mybir.dt.float32
_BF16 = mybir.dt.bfloat16
_BF16_NP = ml_dtypes.bfloat16


def _sgn1d(pos, b2, b1):
    """Composite 2-level Haar sign for position pos in 0..3 (+-1)."""
    s1 = 1.0 if b1 == 0 else (1.0 - 2.0 * (pos % 2))
    s2 = 1.0 if b2 == 0 else (1.0 - 2.0 * (pos // 2))
    return s1 * s2


def _build_signs():
    """bf16 sign matrices including the global 1/64 scale (exact in bf16).

    Rows k = hh*4 + dt (h-major so the load's HBM AP outer dim is h).
    Cols m = (T2*2+H2)*32 + y'*4 + (T1*2+H1)   (y' = hh//4).
    s32 [128,128] / s24 [96,128]: t>=1.  t32 [32,128] / t24 [24,128]:
    t=0 (frame 0 repeated 4x -> only T2=T1=0 subbands, weight 4).
    """
    def mk(nh, t0):
        k = nh if t0 else 4 * nh
        s = np.zeros((k, 128), dtype=np.float32)
        for hh in range(nh):
            yp, hp = hh // 4, hh % 4
            for t2 in range(2):
                for h2 in range(2):
                    for t1 in range(2):
                        for h1 in range(2):
                            col = (t2 * 2 + h2) * 32 + yp * 4 + (t1 * 2 + h1)
                            sh = _sgn1d(hp, h2, h1)
                            if t0:
                                if t2 == 0 and t1 == 0:
                                    s[hh, col] = 4.0 * sh / 64.0
                            else:
                                for dt in range(4):
                                    st = _sgn1d(dt, t2, t1)
                                    s[hh * 4 + dt, col] = st * sh / 64.0
        return s.astype(_BF16_NP)

    return mk(32, False), mk(24, False), mk(32, True), mk(24, True)


def _build_nc():
    nc = bacc.Bacc(
        "TRN2", target_bir_lowering=False, debug=False, num_devices=N_CORES
    )
    # host packs x as [T, H, C, W] so (c, w) is one contiguous 4224B run
    x = nc.dram_tensor(
        "x", [T_IN, H_SH, C, W_IN], _BF16, kind="ExternalInput"
    ).ap()
    sg = {}
    for nm, shp in [
        ("s32", [128, 128]), ("s24", [96, 128]),
        ("t32", [32, 128]), ("t24", [24, 128]),
        ("s32n", [128, 128]), ("s24n", [96, 128]),
        ("t32n", [32, 128]), ("t24n", [24, 128]),
    ]:
        sg[nm] = nc.dram_tensor(nm, shp, _BF16, kind="ExternalInput").ap()
    out = nc.dram_tensor(
        "out", [192, T_OUT, Y_SH, X_OUT], _F32, kind="ExternalOutput"
    ).ap()

    # Store view: ch = 48*TH2 + 24*W2 + i, i = 6*TH1 + 3*W1 + c.
    # Per (TH2, W2) store: HBM dims [y', i(24) @ ch-stride, x] -> 3 dims.
    # (outer dim is y' (<=8): HWDGE puts block j on engine 64+j, so HWDGE
    # stores only reach engines 64-71; SWDGE round-robins all 16 evenly.)
    o_v = out.rearrange("(a b i) t y x -> a b t y i x", a=4, b=2, i=24)

    with tile.TileContext(nc) as tc:
        with (
            tc.tile_pool(name="signs", bufs=1) as sgp,
            tc.tile_pool(name="rhs", bufs=4) as rhp,
            tc.tile_pool(name="even", bufs=4) as evp,
            tc.tile_pool(name="outp", bufs=4) as otp,
            tc.tile_pool(name="psum", bufs=2, space="PSUM") as psp,
        ):
            st = {}
            for nm in ("s32", "s24", "t32", "t24", "s32n", "s24n", "t32n", "t24n"):
                t_ = sgp.tile(list(sg[nm].shape), _BF16, tag=nm)
                nc.sync.dma_start(out=t_, in_=sg[nm])
                st[nm] = t_

            store_i = 0
            for t in range(T_OUT):
                for ci, (h0, nh) in enumerate(CHUNKS):
                    ny = nh // 4
                    kdim = nh if t == 0 else 4 * nh
                    if t == 0:
                        lp = st["t32"] if nh == 32 else st["t24"]
                        ln = st["t32n"] if nh == 32 else st["t24n"]
                    else:
                        lp = st["s32"] if nh == 32 else st["s24"]
                        ln = st["s32n"] if nh == 32 else st["s24n"]

                    # one load per (t, chunk): partitions k = hh*4 + dt
                    rhs = rhp.tile([128, C, W_IN], _BF16, tag="rhs")
                    if t == 0:
                        src = x[0, h0 : h0 + nh].rearrange("h c w -> h (c w)")
                    else:
                        src = x[4 * t - 3 : 4 * t + 1, h0 : h0 + nh].rearrange(
                            "t h c w -> h t (c w)"
                        )
                    nc.sync.dma_start(
                        out=rhs[:kdim].rearrange("k c w -> k (c w)"), in_=src
                    )

                    rv = rhs.rearrange("k c (xp par) -> k c xp par", par=2)

                    # W-level-1 sums in PE: ps_s[m, c, x'] = S @ (even+odd).
                    # Each c gets a full 512-f32 PSUM bank so accumulation
                    # groups never straddle bank boundaries.
                    ps_s = psp.tile([128, C, 512], _F32, tag="ps")
                    for c in range(C):
                        nc.tensor.matmul(
                            ps_s[:, c, :XH], lp, rv[:kdim, c, :, 0],
                            start=True, stop=False,
                        )
                        nc.tensor.matmul(
                            ps_s[:, c, :XH], lp, rv[:kdim, c, :, 1],
                            start=False, stop=True,
                        )

                    # W-level-2 from sums -> out tile (w2, w1, c, x)
                    ot = otp.tile([128, 2, 2, C, X_OUT], _F32, tag="ot")
                    sv = ps_s.rearrange("m c (xx par) -> m c xx par", par=2)
                    se = evp.tile([128, C, X_OUT], _F32, tag="ev")
                    nc.scalar.copy(out=se, in_=sv[:, :, :X_OUT, 0])
                    nc.vector.tensor_add(
                        out=ot[:, 0, 0], in0=se, in1=sv[:, :, :X_OUT, 1]
                    )
                    nc.vector.tensor_sub(
                        out=ot[:, 1, 0], in0=se, in1=sv[:, :, :X_OUT, 1]
                    )

                    # W-level-1 diffs in PE: ps_d = S @ even + (-S) @ odd
                    ps_d = psp.tile([128, C, 512], _F32, tag="ps")
                    for c in range(C):
                        nc.tensor.matmul(
                            ps_d[:, c, :XH], lp, rv[:kdim, c, :, 0],
                            start=True, stop=False,
                        )
                    for c in range(C):
                        nc.tensor.matmul(
                            ps_d[:, c, :XH], ln, rv[:kdim, c, :, 1],
                            start=False, stop=True,
                        )

                    dv = ps_d.rearrange("m c (xx par) -> m c xx par", par=2)
                    de = evp.tile([128, C, X_OUT], _F32, tag="ev")
                    nc.vector.tensor_copy(out=de, in_=dv[:, :, :X_OUT, 0])
                    nc.vector.tensor_add(
                        out=ot[:, 0, 1], in0=de, in1=dv[:, :, :X_OUT, 1]
                    )
                    nc.vector.tensor_sub(
                        out=ot[:, 1, 1], in0=de, in1=dv[:, :, :X_OUT, 1]
                    )

                    # stores: one per (TH2, W2). SWDGE (gpsimd) takes ~2/3
                    # for even 16-engine spread (Q7 ~1us/store caps it);
                    # sync/scalar HWDGE take the rest on engines 64-71.
                    y0 = h0 // 4
                    for th2 in range(4):
                        for w2 in range(2):
                            dst = o_v[th2, w2, t, y0 : y0 + ny]
                            src_s = ot[th2 * 32 : th2 * 32 + 4 * ny, w2]
                            if ny == 6:
                                eng = nc.gpsimd
                            else:
                                eng = [nc.gpsimd, nc.sync, nc.gpsimd,
                                       nc.scalar][store_i % 4]
                                store_i += 1
                            eng.dma_start(out=dst, in_=src_s)

    nc.compile()
    return nc


_NC_CACHE = None


def _prep_inputs(hs):
    """Shard along H, convert to single bf16, pack as [T, H, C, W]."""
    s32, s24, t32, t24 = _build_signs()
    base = {
        "s32": s32, "s24": s24, "t32": t32, "t24": t24,
        "s32n": -s32, "s24n": -s24, "t32n": -t32, "t24n": -t24,
    }
    in_maps = []
    for k in range(N_CORES):
        xk = hs[0, :, :, k * H_SH : (k + 1) * H_SH, :]  # [C, T, H, W]
        xk = np.ascontiguousarray(
            xk.transpose(1, 2, 0, 3)
        ).astype(_BF16_NP)                               # [T, H, C, W]
        m = dict(base)
        m["x"] = xk
        in_maps.append(m)
    return in_maps


def kernel(hidden_states: np.ndarray) -> np.ndarray:
    global _NC_CACHE
    if _NC_CACHE is None:
        _NC_CACHE = _build_nc()
    nc = _NC_CACHE

    hs = np.asarray(hidden_states, dtype=np.float32)
    assert hs.shape == (1, C, T_IN, H_IN, W_IN), hs.shape
    in_maps = _prep_inputs(hs)

    res = run_bass_kernel_spmd(nc, in_maps, core_ids=list(range(N_CORES)))

    out = np.empty((1, 192, T_OUT, H_IN // 4, X_OUT), dtype=np.float32)
    for k in range(N_CORES):
        out[0, :, :, k * Y_SH : (k + 1) * Y_SH, :] = res.results[k]["out"]
    return out


# revision 19
# speedup vs baseline: 2.1073x; 1.0122x over previous
"""Trainium2 Bass kernel for nn_CosmosPatcher3d.

Computes the Cosmos 3D Haar wavelet patcher: input [1,3,33,704,704] fp32,
temporal causal pad (first frame repeated 4x -> 36 frames), then two full
3D Haar DWT levels. Equivalent to a separable +-1 Hadamard transform over
4x4x4 blocks scaled by 1/64, producing [1,192,9,176,176] fp32 with channel
layout ch = 96*T2 + 48*H2 + 24*W2 + 12*T1 + 6*H1 + 3*W1 + c.

v4 strategy (8 NeuronCores, shard along H: 704 = 8*88):
- Host converts x to single bf16 (err ~2e-3 << 2e-2 budget) packed as
  [T, H, C, W] so one 3-dim DMA per (t, chunk) loads all channels with
  4224B descriptors, HBM outer dim = h (24..32 blocks -> 16 engines).
- TensorE: fused T+H transform AND W-level-1 butterfly: sums via lhsT=S
  on even/odd stride-2 rhs slices accumulated in PSUM; diffs via S then
  -S. M = th2*32 + y'*4 + th1. Each c gets a full PSUM bank.
- W-level-2: ScalarE/VectorE copy the even-parity PSUM slice to SBUF
  (TensorTensor allows only one PSUM operand), VectorE does add/sub.
- Out tile free layout (w1, c, w2, x) so the store's SBUF-side strides
  (th1@2112, w1@1056, c@352) nest uniformly -> 3-dim AP with HBM outer
  dim i(24): HWDGE spreads stores over all 16 engines. SWDGE (gpsimd)
  takes a share for even balance.
"""

import ml_dtypes
import numpy as np

import concourse.bacc as bacc
import concourse.mybir as mybir
import concourse.tile as tile
from concourse.bass_utils import run_bass_kernel_spmd

N_CORES = 8
C = 3            # input channels
T_IN = 33        # input frames
H_IN = 704       # input height (global)
W_IN = 704       # input width
H_SH = H_IN // N_CORES      # 88 input rows per core
T_OUT = 9
Y_SH = H_SH // 4            # 22 output rows per core
X_OUT = W_IN // 4           # 176
XH = W_IN // 2              # 352 = level-1 output width
CHUNKS = [(0, 32), (32, 32), (64, 24)]

_F32 = mybir.dt.float32
_BF16 = mybir.dt.bfloat16
_BF16_NP = ml_dtypes.bfloat16


def _sgn1d(pos, b2, b1):
    """Composite 2-level Haar sign for position pos in 0..3 (+-1)."""
    s1 = 1.0 if b1 == 0 else (1.0 - 2.0 * (pos % 2))
    s2 = 1.0 if b2 == 0 else (1.0 - 2.0 * (pos // 2))
    return s1 * s2


def _build_signs():
    """bf16 sign matrices including the global 1/64 scale (exact in bf16).

    Rows k = hh*4 + dt (h-major so the load's HBM AP outer dim is h).
    Cols m = (T2*2+H2)*32 + y'*4 + (T1*2+H1)   (y' = hh//4).
    s32 [128,128] / s24 [96,128]: t>=1.  t32 [32,128] / t24 [24,128]:
    t=0 (frame 0 repeated 4x -> only T2=T1=0 subbands, weight 4).
    """
    def mk(nh, t0):
        k = nh if t0 else 4 * nh
        s = np.zeros((k, 128), dtype=np.float32)
        for hh in range(nh):
            yp, hp = hh // 4, hh % 4
            for t2 in range(2):
                for h2 in range(2):
                    for t1 in range(2):
                        for h1 in range(2):
                            col = (t2 * 2 + h2) * 32 + yp * 4 + (t1 * 2 + h1)
                            sh = _sgn1d(hp, h2, h1)
                            if t0:
                                if t2 == 0 and t1 == 0:
                                    s[hh, col] = 4.0 * sh / 64.0
                            else:
                                for dt in range(4):
                                    st = _sgn1d(dt, t2, t1)
                                    s[hh * 4 + dt, col] = st * sh / 64.0
        return s.astype(_BF16_NP)

    return mk(32, False), mk(24, False), mk(32, True), mk(24, True)


def _build_nc():
    nc = bacc.Bacc(
        "TRN2", target_bir_lowering=False, debug=False, num_devices=N_CORES
    )
    # host packs x as [T, H, C, 2, W/2] (W even/odd deinterleaved) so the
    # level-1 matmul rhs slices are contiguous and (c, par, w) is one
    # contiguous 4224B run per (t', h) for the load
    x = nc.dram_tensor(
        "x", [T_IN, H_SH, C, 2, XH], _BF16, kind="ExternalInput"
    ).ap()
    sg = {}
    for nm, shp in [
        ("s32", [128, 128]), ("s24", [96, 128]),
        ("t32", [32, 128]), ("t24", [24, 128]),
        ("s32n", [128, 128]), ("s24n", [96, 128]),
        ("t32n", [32, 128]), ("t24n", [24, 128]),
    ]:
        sg[nm] = nc.dram_tensor(nm, shp, _BF16, kind="ExternalInput").ap()
    out = nc.dram_tensor(
        "out", [192, T_OUT, Y_SH, X_OUT], _F32, kind="ExternalOutput"
    ).ap()

    # Store view: ch = 48*TH2 + 24*W2 + i, i = 6*TH1 + 3*W1 + c.
    # Per (TH2, W2) store: HBM dims [y', i(24) @ ch-stride, x] -> 3 dims.
    # (outer dim is y' (<=8): HWDGE puts block j on engine 64+j, so HWDGE
    # stores only reach engines 64-71; SWDGE round-robins all 16 evenly.)
    o_v = out.rearrange("(a b i) t y x -> a b t y i x", a=4, b=2, i=24)

    with tile.TileContext(nc) as tc:
        with (
            tc.tile_pool(name="signs", bufs=1) as sgp,
            tc.tile_pool(name="rhs", bufs=6) as rhp,
            tc.tile_pool(name="even", bufs=6) as evp,
            tc.tile_pool(name="outp", bufs=6) as otp,
            tc.tile_pool(name="psum", bufs=2, space="PSUM") as psp,
        ):
            st = {}
            for nm in ("s32", "s24", "t32", "t24", "s32n", "s24n", "t32n", "t24n"):
                t_ = sgp.tile(list(sg[nm].shape), _BF16, tag=nm)
                nc.sync.dma_start(out=t_, in_=sg[nm])
                st[nm] = t_

            store_i = 0
            for t in range(T_OUT):
                for ci, (h0, nh) in enumerate(CHUNKS):
                    ny = nh // 4
                    kdim = nh if t == 0 else 4 * nh
                    if t == 0:
                        lp = st["t32"] if nh == 32 else st["t24"]
                        ln = st["t32n"] if nh == 32 else st["t24n"]
                    else:
                        lp = st["s32"] if nh == 32 else st["s24"]
                        ln = st["s32n"] if nh == 32 else st["s24n"]

                    # one load per (t, chunk): partitions k = hh*4 + dt
                    rv = rhp.tile([128, C, 2, XH], _BF16, tag="rhs")
                    if t == 0:
                        src = x[0, h0 : h0 + nh].rearrange(
                            "h c p w -> h (c p w)"
                        )
                    else:
                        src = x[4 * t - 3 : 4 * t + 1, h0 : h0 + nh].rearrange(
                            "t h c p w -> h t (c p w)"
                        )
                    nc.sync.dma_start(
                        out=rv[:kdim].rearrange("k c p w -> k (c p w)"),
                        in_=src,
                    )

                    # W-level-1 sums in PE: ps_s[m, c, x'] = S @ (even+odd).
                    # Each c gets a full 512-f32 PSUM bank so accumulation
                    # groups never straddle bank boundaries.
                    ps_s = psp.tile([128, C, 512], _F32, tag="ps")
                    for c in range(C):
                        nc.tensor.matmul(
                            ps_s[:, c, :XH], lp, rv[:kdim, c, 0],
                            start=True, stop=False,
                        )
                        nc.tensor.matmul(
                            ps_s[:, c, :XH], lp, rv[:kdim, c, 1],
                            start=False, stop=True,
                        )

                    # W-level-2 from sums -> out tile (w2, w1, c, x)
                    ot = otp.tile([128, 2, 2, C, X_OUT], _F32, tag="ot")
                    sv = ps_s.rearrange("m c (xx par) -> m c xx par", par=2)
                    se = evp.tile([128, C, X_OUT], _F32, tag="ev")
                    nc.scalar.copy(out=se, in_=sv[:, :, :X_OUT, 0])
                    nc.vector.tensor_add(
                        out=ot[:, 0, 0], in0=se, in1=sv[:, :, :X_OUT, 1]
                    )
                    nc.vector.tensor_sub(
                        out=ot[:, 1, 0], in0=se, in1=sv[:, :, :X_OUT, 1]
                    )

                    # W-level-1 diffs in PE: ps_d = S @ even + (-S) @ odd
                    ps_d = psp.tile([128, C, 512], _F32, tag="ps")
                    for c in range(C):
                        nc.tensor.matmul(
                            ps_d[:, c, :XH], lp, rv[:kdim, c, 0],
                            start=True, stop=False,
                        )
                    for c in range(C):
                        nc.tensor.matmul(
                            ps_d[:, c, :XH], ln, rv[:kdim, c, 1],
                            start=False, stop=True,
                        )

                    dv = ps_d.rearrange("m c (xx par) -> m c xx par", par=2)
                    de = evp.tile([128, C, X_OUT], _F32, tag="ev")
                    nc.vector.tensor_copy(out=de, in_=dv[:, :, :X_OUT, 0])
                    nc.vector.tensor_add(
                        out=ot[:, 0, 1], in0=de, in1=dv[:, :, :X_OUT, 1]
                    )
                    nc.vector.tensor_sub(
                        out=ot[:, 1, 1], in0=de, in1=dv[:, :, :X_OUT, 1]
                    )

                    # stores: one per (TH2, W2). SWDGE (gpsimd) takes ~2/3
                    # for even 16-engine spread (Q7 ~1us/store caps it);
                    # sync/scalar HWDGE take the rest on engines 64-71.
                    y0 = h0 // 4
                    for th2 in range(4):
                        for w2 in range(2):
                            dst = o_v[th2, w2, t, y0 : y0 + ny]
                            src_s = ot[th2 * 32 : th2 * 32 + 4 * ny, w2]
                            if ny == 6:
                                eng = nc.gpsimd
                            else:
                                eng = [nc.gpsimd, nc.sync, nc.gpsimd,
                                       nc.scalar][store_i % 4]
                                store_i += 1
                            eng.dma_start(out=dst, in_=src_s)

    nc.compile()
    return nc


_NC_CACHE = None


def _prep_inputs(hs):
    """Shard along H, convert to single bf16, pack as [T, H, C, W]."""
    s32, s24, t32, t24 = _build_signs()
    base = {
        "s32": s32, "s24": s24, "t32": t32, "t24": t24,
        "s32n": -s32, "s24n": -s24, "t32n": -t32, "t24n": -t24,
    }
    in_maps = []
    for k in range(N_CORES):
        xk = hs[0, :, :, k * H_SH : (k + 1) * H_SH, :]  # [C, T, H, W]
        xk = xk.transpose(1, 2, 0, 3)                    # [T, H, C, W]
        xk = xk.reshape(T_IN, H_SH, C, XH, 2).transpose(0, 1, 2, 4, 3)
        xk = np.ascontiguousarray(xk).astype(_BF16_NP)   # [T, H, C, 2, W/2]
        m = dict(base)
        m["x"] = xk
        in_maps.append(m)
    return in_maps


def kernel(hidden_states: np.ndarray) -> np.ndarray:
    global _NC_CACHE
    if _NC_CACHE is None:
        _NC_CACHE = _build_nc()
    nc = _NC_CACHE

    hs = np.asarray(hidden_states, dtype=np.float32)
    assert hs.shape == (1, C, T_IN, H_IN, W_IN), hs.shape
    in_maps = _prep_inputs(hs)

    res = run_bass_kernel_spmd(nc, in_maps, core_ids=list(range(N_CORES)))

    out = np.empty((1, 192, T_OUT, H_IN // 4, X_OUT), dtype=np.float32)
    for k in range(N_CORES):
        out[0, :, :, k * Y_SH : (k + 1) * Y_SH, :] = res.results[k]["out"]
    return out


# revision 23
# speedup vs baseline: 2.4210x; 1.1488x over previous
"""Trainium2 Bass kernel for nn_CosmosPatcher3d.

Computes the Cosmos 3D Haar wavelet patcher: input [1,3,33,704,704] fp32,
temporal causal pad (first frame repeated 4x -> 36 frames), then two full
3D Haar DWT levels. Equivalent to a separable +-1 Hadamard transform over
4x4x4 blocks scaled by 1/64, producing [1,192,9,176,176] fp32 with channel
layout ch = 96*T2 + 48*H2 + 24*W2 + 12*T1 + 6*H1 + 3*W1 + c.

v4 strategy (8 NeuronCores, shard along H: 704 = 8*88):
- Host converts x to single bf16 (err ~2e-3 << 2e-2 budget) packed as
  [T, H, C, W] so one 3-dim DMA per (t, chunk) loads all channels with
  4224B descriptors, HBM outer dim = h (24..32 blocks -> 16 engines).
- TensorE: fused T+H transform AND W-level-1 butterfly: sums via lhsT=S
  on even/odd stride-2 rhs slices accumulated in PSUM; diffs via S then
  -S. M = th2*32 + y'*4 + th1. Each c gets a full PSUM bank.
- W-level-2: ScalarE/VectorE copy the even-parity PSUM slice to SBUF
  (TensorTensor allows only one PSUM operand), VectorE does add/sub.
- Out tile free layout (w1, c, w2, x) so the store's SBUF-side strides
  (th1@2112, w1@1056, c@352) nest uniformly -> 3-dim AP with HBM outer
  dim i(24): HWDGE spreads stores over all 16 engines. SWDGE (gpsimd)
  takes a share for even balance.
"""

import ml_dtypes
import numpy as np

import concourse.bacc as bacc
import concourse.mybir as mybir
import concourse.tile as tile
from concourse.bass_utils import run_bass_kernel_spmd

N_CORES = 8
C = 3            # input channels
T_IN = 33        # input frames
H_IN = 704       # input height (global)
W_IN = 704       # input width
H_SH = H_IN // N_CORES      # 88 input rows per core
T_OUT = 9
Y_SH = H_SH // 4            # 22 output rows per core
X_OUT = W_IN // 4           # 176
XH = W_IN // 2              # 352 = level-1 output width
CHUNKS = [(0, 32), (32, 32), (64, 24)]

_F32 = mybir.dt.float32
_BF16 = mybir.dt.bfloat16
_BF16_NP = ml_dtypes.bfloat16


def _sgn1d(pos, b2, b1):
    """Composite 2-level Haar sign for position pos in 0..3 (+-1)."""
    s1 = 1.0 if b1 == 0 else (1.0 - 2.0 * (pos % 2))
    s2 = 1.0 if b2 == 0 else (1.0 - 2.0 * (pos // 2))
    return s1 * s2


def _build_signs():
    """bf16 sign matrices including the global 1/64 scale (exact in bf16).

    Rows k = hh*4 + dt (h-major so the load's HBM AP outer dim is h).
    Cols m = (T2*2+H2)*32 + y'*4 + (T1*2+H1)   (y' = hh//4).
    s32 [128,128] / s24 [96,128]: t>=1.  t32 [32,128] / t24 [24,128]:
    t=0 (frame 0 repeated 4x -> only T2=T1=0 subbands, weight 4).
    """
    def mk(nh, t0):
        k = nh if t0 else 4 * nh
        s = np.zeros((k, 128), dtype=np.float32)
        for hh in range(nh):
            yp, hp = hh // 4, hh % 4
            for t2 in range(2):
                for h2 in range(2):
                    for t1 in range(2):
                        for h1 in range(2):
                            col = (t2 * 2 + h2) * 32 + yp * 4 + (t1 * 2 + h1)
                            sh = _sgn1d(hp, h2, h1)
                            if t0:
                                if t2 == 0 and t1 == 0:
                                    s[hh, col] = 4.0 * sh / 64.0
                            else:
                                for dt in range(4):
                                    st = _sgn1d(dt, t2, t1)
                                    s[hh * 4 + dt, col] = st * sh / 64.0
        return s.astype(_BF16_NP)

    return mk(32, False), mk(24, False), mk(32, True), mk(24, True)


def _build_nc():
    nc = bacc.Bacc(
        "TRN2", target_bir_lowering=False, debug=False, num_devices=N_CORES
    )
    # host packs x as [T, H, C, 2, W/2] (W even/odd deinterleaved) so the
    # level-1 matmul rhs slices are contiguous and (c, par, w) is one
    # contiguous 4224B run per (t', h) for the load
    x = nc.dram_tensor(
        "x", [T_IN, H_SH, C, 2, XH], _BF16, kind="ExternalInput"
    ).ap()
    sg = {}
    for nm, shp in [
        ("s32", [128, 128]), ("s24", [96, 128]),
        ("t32", [32, 128]), ("t24", [24, 128]),
        ("s32n", [128, 128]), ("s24n", [96, 128]),
        ("t32n", [32, 128]), ("t24n", [24, 128]),
    ]:
        sg[nm] = nc.dram_tensor(nm, shp, _BF16, kind="ExternalInput").ap()
    # Packed output: one [128, 2112] f32 tile per (t, chunk), stored as a
    # single contiguous 1.08MB DMA (outer dim 128 -> all 16 engines, 8448B
    # descriptors). The host unpacks to the [192, 9, 176, 176] layout.
    out = nc.dram_tensor(
        "out", [T_OUT * len(CHUNKS), 128, 2112], _F32, kind="ExternalOutput"
    ).ap()

    with tile.TileContext(nc) as tc:
        with (
            tc.tile_pool(name="signs", bufs=1) as sgp,
            tc.tile_pool(name="rhs", bufs=6) as rhp,
            tc.tile_pool(name="even", bufs=6) as evp,
            tc.tile_pool(name="outp", bufs=6) as otp,
            tc.tile_pool(name="psum", bufs=2, space="PSUM") as psp,
        ):
            st = {}
            for nm in ("s32", "s24", "t32", "t24", "s32n", "s24n", "t32n", "t24n"):
                t_ = sgp.tile(list(sg[nm].shape), _BF16, tag=nm)
                nc.sync.dma_start(out=t_, in_=sg[nm])
                st[nm] = t_

            store_i = 0
            for t in range(T_OUT):
                for ci, (h0, nh) in enumerate(CHUNKS):
                    ny = nh // 4
                    kdim = nh if t == 0 else 4 * nh
                    if t == 0:
                        lp = st["t32"] if nh == 32 else st["t24"]
                        ln = st["t32n"] if nh == 32 else st["t24n"]
                    else:
                        lp = st["s32"] if nh == 32 else st["s24"]
                        ln = st["s32n"] if nh == 32 else st["s24n"]

                    # one load per (t, chunk): partitions k = hh*4 + dt
                    rv = rhp.tile([128, C, 2, XH], _BF16, tag="rhs")
                    if t == 0:
                        src = x[0, h0 : h0 + nh].rearrange(
                            "h c p w -> h (c p w)"
                        )
                    else:
                        src = x[4 * t - 3 : 4 * t + 1, h0 : h0 + nh].rearrange(
                            "t h c p w -> h t (c p w)"
                        )
                    nc.sync.dma_start(
                        out=rv[:kdim].rearrange("k c p w -> k (c p w)"),
                        in_=src,
                    )

                    # W-level-1 sums in PE: ps_s[m, c, x'] = S @ (even+odd).
                    # Each c gets a full 512-f32 PSUM bank so accumulation
                    # groups never straddle bank boundaries.
                    ps_s = psp.tile([128, C, 512], _F32, tag="ps")
                    for c in range(C):
                        nc.tensor.matmul(
                            ps_s[:, c, :XH], lp, rv[:kdim, c, 0],
                            start=True, stop=False,
                        )
                        nc.tensor.matmul(
                            ps_s[:, c, :XH], lp, rv[:kdim, c, 1],
                            start=False, stop=True,
                        )

                    # W-level-2 from sums -> out tile (w2, w1, c, x)
                    ot = otp.tile([128, 2, 2, C, X_OUT], _F32, tag="ot")
                    sv = ps_s.rearrange("m c (xx par) -> m c xx par", par=2)
                    se = evp.tile([128, C, X_OUT], _F32, tag="ev")
                    nc.scalar.copy(out=se, in_=sv[:, :, :X_OUT, 0])
                    nc.vector.tensor_add(
                        out=ot[:, 0, 0], in0=se, in1=sv[:, :, :X_OUT, 1]
                    )
                    nc.vector.tensor_sub(
                        out=ot[:, 1, 0], in0=se, in1=sv[:, :, :X_OUT, 1]
                    )

                    # W-level-1 diffs in PE: ps_d = S @ even + (-S) @ odd
                    ps_d = psp.tile([128, C, 512], _F32, tag="ps")
                    for c in range(C):
                        nc.tensor.matmul(
                            ps_d[:, c, :XH], lp, rv[:kdim, c, 0],
                            start=True, stop=False,
                        )
                    for c in range(C):
                        nc.tensor.matmul(
                            ps_d[:, c, :XH], ln, rv[:kdim, c, 1],
                            start=False, stop=True,
                        )

                    # d-path: GpSimd cannot read PSUM, so ACT escapes both
                    # parities; GpSimd adds (SBUF+SBUF), DVE subs (PSUM ok)
                    dv = ps_d.rearrange("m c (xx par) -> m c xx par", par=2)
                    de = evp.tile([128, C, X_OUT], _F32, tag="ev")
                    do_ = evp.tile([128, C, X_OUT], _F32, tag="ev")
                    nc.scalar.copy(out=de, in_=dv[:, :, :X_OUT, 0])
                    nc.scalar.copy(out=do_, in_=dv[:, :, :X_OUT, 1])
                    nc.gpsimd.tensor_add(out=ot[:, 0, 1], in0=de, in1=do_)
                    nc.vector.tensor_sub(
                        out=ot[:, 1, 1], in0=de, in1=dv[:, :, :X_OUT, 1]
                    )

                    # one packed 1.08MB store per (t, chunk) on sync HWDGE
                    nc.sync.dma_start(
                        out=out[t * len(CHUNKS) + ci],
                        in_=ot.rearrange("m a b c x -> m (a b c x)"),
                    )

    nc.compile()
    return nc


_NC_CACHE = None


def _prep_inputs(hs):
    """Shard along H, convert to single bf16, pack as [T, H, C, W]."""
    s32, s24, t32, t24 = _build_signs()
    base = {
        "s32": s32, "s24": s24, "t32": t32, "t24": t24,
        "s32n": -s32, "s24n": -s24, "t32n": -t32, "t24n": -t24,
    }
    in_maps = []
    for k in range(N_CORES):
        xk = hs[0, :, :, k * H_SH : (k + 1) * H_SH, :]  # [C, T, H, W]
        xk = xk.transpose(1, 2, 0, 3)                    # [T, H, C, W]
        xk = xk.reshape(T_IN, H_SH, C, XH, 2).transpose(0, 1, 2, 4, 3)
        xk = np.ascontiguousarray(xk).astype(_BF16_NP)   # [T, H, C, 2, W/2]
        m = dict(base)
        m["x"] = xk
        in_maps.append(m)
    return in_maps


def kernel(hidden_states: np.ndarray) -> np.ndarray:
    global _NC_CACHE
    if _NC_CACHE is None:
        _NC_CACHE = _build_nc()
    nc = _NC_CACHE

    hs = np.asarray(hidden_states, dtype=np.float32)
    assert hs.shape == (1, C, T_IN, H_IN, W_IN), hs.shape
    in_maps = _prep_inputs(hs)

    res = run_bass_kernel_spmd(nc, in_maps, core_ids=list(range(N_CORES)))

    out = np.empty((1, 192, T_OUT, H_IN // 4, X_OUT), dtype=np.float32)
    # unpack [27, 128, 2112] -> [192, 9, 22, 176] per core:
    # m = th2*32 + yp*4 + th1, f = w2*1056 + w1*528 + c*176 + x,
    # ch = 48*th2 + 24*w2 + 6*th1 + 3*w1 + c
    ov = out[0].reshape(4, 2, 4, 2, C, T_OUT, H_IN // 4, X_OUT)
    for k in range(N_CORES):
        arr = np.asarray(res.results[k]["out"]).reshape(
            T_OUT, len(CHUNKS), 4, 8, 4, 2, 2, C, X_OUT
        )  # [t, ci, th2, yp, th1, w2, w1, c, x]
        for ci, (h0, nh) in enumerate(CHUNKS):
            ny, y0 = nh // 4, h0 // 4
            ov[:, :, :, :, :, :, k * Y_SH + y0 : k * Y_SH + y0 + ny, :] = (
                arr[:, ci, :, :ny].transpose(1, 4, 3, 5, 6, 0, 2, 7)
            )
    return out
